# revision 34
# baseline (speedup 1.0000x reference)
"""Tensor-parallel fused attention kernel for Trainium2 (8 NeuronCores).

Problem: x[2,2048,4096] -> QKV proj (GQA 32q/8kv heads, head_dim 128) ->
RoPE -> causal attention -> out proj, all f32 I/O.

Sharding: tensor-parallel over heads. Core c gets q heads 4c..4c+3 and
kv head c (w_qkv rows), plus the matching 512 columns of w_o. x is
replicated (transposed + bf16-cast on host). Each core emits a partial
y [4096, 4096]; the host sums the 8 partials.

v3 layout strategy:
- Phase 1 (QKV proj): WEIGHT-stationary. Per 256-token block, six
  128-feature stationary groups (q0..q3, k, v) with x as the moving
  operand emit q/k directly TRANSPOSED ([head_dim, token] attention
  layout) in PSUM. RoPE happens during PSUM evacuation on the DVE
  (cross-partition reads of the psum halves; sign baked into the sin
  table), writing qkT straight to SBUF — zero DMA-xbar transposes.
  v is evacuated via two PE identity-transposes into token-natural
  layout. DMA issue is spread across the three hardware queues (x on
  Sync, weights/tables on Scalar, w_o on GpSimd) so the PE never waits
  on a serialized descriptor queue.
- Phase 2+3 fused (unchanged from v2): per 512-query block, scores^T =
  k_chunk.T @ q with causal column-range restriction on diagonal
  chunks; exp on ScalarE; denominator via all-ones matmul +
  reciprocal; PV runs one chunk behind scores and out-projection
  matmul groups drain from a FIFO between attention slots. y writes
  alternate Sync/GpSimd queues; the final drain splits copies across
  Scalar+Vector and DMAs across queues to shorten the tail.
"""

import numpy as np
import ml_dtypes

import concourse.bass as bass
import concourse.mybir as mybir
import concourse.tile as tile
from concourse import bacc
from concourse.bass_utils import run_bass_kernel_spmd

F32 = mybir.dt.float32
F32R = mybir.dt.float32r
BF16 = mybir.dt.bfloat16
AF = mybir.ActivationFunctionType
BF = ml_dtypes.bfloat16

# Model dims (hardcoded per contract)
B, S, D = 2, 2048, 4096
H, KV, DH = 32, 8, 128
T = B * S                     # 4096 tokens, batch-major
N_CORES = 8
HPC = H // N_CORES            # 4 q heads per core
QKV_ROWS = HPC * DH + 2 * DH  # 768 rows of w_qkv per core
WO_COLS = HPC * DH            # 512 w_o columns per core
SCALE = 1.0 / np.sqrt(DH)

KCH = D // 128                # 32 contraction chunks
NG = 6                        # stationary feature groups: q0..q3, k, v
TB = 256                      # phase-1 token block
NB = T // TB                  # 16 blocks
SQ = 512                      # phase-2 q block
N_QB = S // SQ                # 4 q blocks per sequence


def _build_nc():
    nc = bacc.Bacc()

    xP = nc.declare_dram_parameter("xP", [NB, 128, KCH * TB], BF16,
                                   isOutput=False)
    wq2p = nc.declare_dram_parameter("wq2", [128, NG, KCH, 128], BF16,
                                     isOutput=False)
    woT = nc.declare_dram_parameter("woT", [WO_COLS, D], BF16, isOutput=False)
    csf = nc.declare_dram_parameter("csf", [64, S], BF16, isOutput=False)
    snf = nc.declare_dram_parameter("snf", [64, S], BF16, isOutput=False)
    maskT = nc.declare_dram_parameter("maskT", [128, 128], BF16,
                                      isOutput=False)
    eye = nc.declare_dram_parameter("eye", [128, 128], BF16, isOutput=False)
    y = nc.declare_dram_parameter("y", [T, D], BF16, isOutput=True)

    woT3 = woT.rearrange("(h p) d -> p h d", p=128)     # [128, 4, 4096]
    y3 = y.rearrange("(tm p) d -> p tm d", p=128)       # [128, 32, 4096]

    with tile.TileContext(nc) as tc:
        with tc.tile_pool(name="persist", bufs=1) as persist:

            # --- persistent tiles --- (mask/eye on the idle gpsimd queue so
            # the sync queue starts on x immediately)
            maskT_t = persist.tile([128, 128], BF16)
            nc.gpsimd.dma_start(maskT_t[:], maskT[:])
            eye_t = persist.tile([128, 128], BF16)
            nc.gpsimd.dma_start(eye_t[:], eye[:])
            ones_bf = persist.tile([128, 128], BF16)
            nc.vector.memset(ones_bf[:], 1.0)
            # wo is DMA'd later (inside phase 1, on the idle gpsimd queue)
            wo = persist.tile([128, HPC, D], BF16)

            # attention-layout q/k storage [DH, T]; v natural [tok, DH]
            qkT = [persist.tile([128, T], BF16, tag=f"qk{m}", name=f"qk{m}")
                   for m in range(5)]
            v_nat = persist.tile([128, T // 128, 128], BF16)

            # ===== Phase 1: QKV projection (w-stationary) + fused RoPE =====
            with tc.tile_pool(name="p1", bufs=3) as p1, \
                 tc.tile_pool(name="p1w", bufs=1) as p1w, \
                 tc.tile_pool(name="p1r", bufs=2) as p1r, \
                 tc.tile_pool(name="p1v", bufs=4) as p1v, \
                 tc.tile_pool(name="psG", bufs=2, space="PSUM") as psG, \
                 tc.tile_pool(name="psT", bufs=2, space="PSUM") as psT:
                wq2 = p1w.tile([128, NG, KCH, 128], BF16, tag="wq")
                # only 64 distinct rows: rope reads them against both psum
                # halves (psum operand exempts the equal-base-partition rule)
                csf_t = p1w.tile([64, S], BF16, tag="csf")
                snf_t = p1w.tile([64, S], BF16, tag="snf")
                # weights + rope tables stream on the Scalar hwdge queue in
                # consumption order, x tiles on the Sync queue: the two never
                # serialize behind each other. Block-0 rope tables and a
                # fine-grained split of the first groups go first so the PE
                # ramp isn't gated on later weights (DMA rings service all
                # outstanding transfers concurrently — issue order is
                # effectively priority).
                nc.scalar.dma_start(csf_t[:, 0:3 * TB], csf[:, 0:3 * TB])
                nc.scalar.dma_start(snf_t[:, 0:3 * TB], snf[:, 0:3 * TB])
                for j in range(8):
                    nc.scalar.dma_start(wq2[:, 0, 4 * j:4 * (j + 1)],
                                        wq2p[:, 0, 4 * j:4 * (j + 1)])
                for g in range(1, NG):
                    nc.scalar.dma_start(wq2[:, g, 0:KCH // 2],
                                        wq2p[:, g, 0:KCH // 2])
                    nc.scalar.dma_start(wq2[:, g, KCH // 2:],
                                        wq2p[:, g, KCH // 2:])
                    if g == 3:
                        nc.scalar.dma_start(csf_t[:, 3 * TB:],
                                            csf[:, 3 * TB:])
                        nc.scalar.dma_start(snf_t[:, 3 * TB:],
                                            snf[:, 3 * TB:])

                pend_v = []   # deferred v transposes: (vstage, bb)

                def flush_v(n):
                    for _ in range(min(n, len(pend_v))):
                        vstage, vb = pend_v.pop(0)
                        for h2 in range(2):
                            pt = psT.tile([128, 128], BF16, tag="tr")
                            nc.tensor.transpose(
                                pt[:], vstage[:, h2 * 128:(h2 + 1) * 128],
                                eye_t[:])
                            nc.scalar.copy(v_nat[:, vb * 2 + h2, :], pt[:])

                xts = {}

                def load_x(bb):
                    xt = p1.tile([128, KCH, TB], BF16, tag="xt")
                    nsub = 16 if bb == 0 else 8
                    kw = KCH // nsub
                    w = KCH * TB // nsub
                    for q in range(nsub):
                        nc.sync.dma_start(
                            xt[:, kw * q:kw * (q + 1), :]
                            .rearrange("p a b -> p (a b)"),
                            xP[bb, :, w * q:w * (q + 1)])
                    xts[bb] = xt

                # first three blocks run as a wavefront ordered by weight
                # arrival (wq streams group-major): the PE always has an
                # unlocked (block, group) task while wq trickles in
                WAVE = [(0, 0), (0, 1), (1, 0), (1, 1), (0, 2), (1, 2),
                        (2, 0), (2, 1), (2, 2), (0, 3), (1, 3), (2, 3),
                        (0, 4), (1, 4), (2, 4), (0, 5), (1, 5), (2, 5)]
                for bb in range(3):
                    load_x(bb)
                tasks = WAVE + [(bb, g) for bb in range(3, NB)
                                for g in range(NG)]
                for ti, (bb, g) in enumerate(tasks):
                    if g == 0 and bb >= 3:
                        load_x(bb)
                        for v in list(xts):
                            if v < bb - 2:
                                del xts[v]
                    if bb == 3 and g == 0:
                        for h in range(HPC):
                            nc.gpsimd.dma_start(wo[:, h, :], woT3[:, h, :])
                    xt = xts[bb]
                    ts = (bb * TB) % S
                    cs_sl = csf_t[:, ts:ts + TB]
                    sn_sl = snf_t[:, ts:ts + TB]
                    tok = slice(bb * TB, (bb + 1) * TB)
                    if True:
                        pg = psG.tile([128, TB], F32, tag="g")
                        for k in range(KCH):
                            nc.tensor.matmul(pg[:], wq2[:, g, k, :],
                                             xt[:, k, :],
                                             start=(k == 0),
                                             stop=(k == KCH - 1))
                        if g in (2, 4):
                            flush_v(1)
                        if g < 5:
                            # RoPE on evacuation (halves on partitions; the
                            # psum operand exempts the equal-base rule):
                            #   lo = p_lo*cs - p_hi*sn ; hi = p_hi*cs + p_lo*sn
                            t1 = p1r.tile([128, TB], F32, tag="t1")
                            t2 = p1r.tile([128, TB], F32, tag="t2")
                            nc.vector.tensor_mul(t1[0:64, :], pg[0:64, :],
                                                 cs_sl)
                            nc.vector.tensor_mul(t1[64:128, :], pg[64:128, :],
                                                 cs_sl)
                            nc.vector.tensor_mul(t2[0:64, :], pg[64:128, :],
                                                 sn_sl)
                            nc.vector.tensor_mul(t2[64:128, :], pg[0:64, :],
                                                 sn_sl)
                            nc.vector.tensor_sub(qkT[g][0:64, tok],
                                                 t1[0:64, :], t2[0:64, :])
                            nc.vector.tensor_add(qkT[g][64:128, tok],
                                                 t1[64:128, :], t2[64:128, :])
                        else:
                            vstage = p1v.tile([128, TB], BF16, tag="vs")
                            nc.scalar.copy(vstage[:], pg[:])
                            pend_v.append((vstage, bb))
                flush_v(len(pend_v))

            # ============ Phase 2+3 fused: attention + out-projection ========
            with tc.tile_pool(name="p2", bufs=3) as p2, \
                 tc.tile_pool(name="p2acc", bufs=2) as p2acc, \
                 tc.tile_pool(name="p2n", bufs=2) as p2n, \
                 tc.tile_pool(name="p2o", bufs=3) as p2o, \
                 tc.tile_pool(name="p2y", bufs=4) as p2y, \
                 tc.tile_pool(name="psS", bufs=4, space="PSUM") as psS, \
                 tc.tile_pool(name="psO", bufs=1, space="PSUM") as psO, \
                 tc.tile_pool(name="psY", bufs=2, space="PSUM") as psY:

                k_t = qkT[4]
                # FIFO of out-projection groups from completed blocks:
                # (outT_tile, tmg, dn, parity)
                pending = []

                def emit_outproj(n, final=False):
                    for _ in range(min(n, len(pending))):
                        outT_tile, tmg, dn, parity = pending.pop(0)
                        # keep the gpsimd (SWDGE) queue out of the last few
                        # groups: its end-of-kernel drain is ~4.5us and must
                        # overlap the sync-queue tail, not extend it
                        tail4 = final and len(pending) < 4
                        py = psY.tile([128, 512], F32, tag="py")
                        for hh in range(HPC):
                            nc.tensor.matmul(
                                py[:], outT_tile[:, hh, (tmg % 4) * 128:
                                                 (tmg % 4 + 1) * 128],
                                wo[:, hh, dn * 512:(dn + 1) * 512],
                                start=(hh == 0), stop=(hh == HPC - 1))
                        if final:
                            # alternate full-width copies between engines on
                            # independent tile tags (one copy+one DMA per
                            # 860ns matmul-group cadence keeps the drain
                            # pipelined with zero backlog)
                            ysb = p2y.tile([128, 512], BF16,
                                           tag="ysbS" if parity else "ysbV")
                            if parity:
                                nc.scalar.copy(ysb[:], py[:])
                            else:
                                nc.vector.tensor_copy(ysb[:], py[:])
                            (nc.sync if tail4 else nc.gpsimd).dma_start(
                                y3[:, tmg, dn * 512:(dn + 1) * 512], ysb[:])
                        else:
                            ysb = p2y.tile([128, 512], BF16, tag="ysb")
                            if parity:
                                nc.scalar.copy(ysb[:], py[:])
                            else:
                                nc.vector.tensor_copy(ysb[:], py[:])
                            if parity:
                                nc.sync.dma_start(
                                    y3[:, tmg, dn * 512:(dn + 1) * 512],
                                    ysb[:])
                            else:
                                nc.gpsimd.dma_start(
                                    y3[:, tmg, dn * 512:(dn + 1) * 512],
                                    ysb[:])

                for b in range(B):
                    tb = b * S
                    for qb in range(N_QB):
                        q0 = tb + qb * SQ
                        nki = 4 * qb + 4
                        outT_blk = p2o.tile([128, HPC, SQ], BF16, tag="outT")
                        # spread the queued out-projection work evenly over
                        # this block's slots: a fixed 2/slot drains early on
                        # big blocks, leaving their tail exp-bound
                        slots = 2 * nki + 2
                        budget0 = len(pending)
                        si = 0

                        def drain_share():
                            # weight the FIFO drain toward later slots: the
                            # tail of each block is exp-latency-bound (short
                            # diagonal matmuls), the head is PE-dense
                            nonlocal si
                            tw = slots * (slots + 1) // 2
                            c1 = budget0 * ((si + 1) * (si + 2) // 2) // tw
                            c0 = budget0 * (si * (si + 1) // 2) // tw
                            emit_outproj(c1 - c0)
                            si += 1
                        for hg in range(2):
                            heads = (2 * hg, 2 * hg + 1)
                            accs = {h: p2acc.tile([128, SQ], F32,
                                                  tag=f"acc{h % 2}",
                                                  name=f"acc{h}")
                                    for h in heads}
                            pos = {h: psO.tile([128, SQ], F32,
                                               tag=f"po{h % 2}",
                                               name=f"po{h}") for h in heads}
                            prev = None
                            for ki in range(nki):
                                dj = ki - 4 * qb
                                off = 128 * dj if dj > 0 else 0
                                ksl = k_t[:, tb + ki * 128:
                                          tb + (ki + 1) * 128]
                                prs = {}
                                for h in heads:
                                    pss = psS.tile([128, SQ], F32, tag="ss")
                                    nc.tensor.matmul(
                                        pss[:, off:], ksl,
                                        qkT[h][:, q0 + off:q0 + SQ],
                                        start=True, stop=True)
                                    pr = p2.tile([128, SQ], BF16,
                                                 tag=f"pr{h % 2}")
                                    nc.scalar.activation(
                                        pr[:, off:], pss[:, off:], AF.Exp,
                                        scale=SCALE)
                                    if dj >= 0:
                                        nc.vector.tensor_mul(
                                            pr[:, off:off + 128],
                                            pr[:, off:off + 128], maskT_t[:])
                                    if ki == 0:
                                        nc.vector.tensor_copy(accs[h][:],
                                                              pr[:])
                                    else:
                                        nc.vector.tensor_add(
                                            accs[h][:, off:],
                                            accs[h][:, off:], pr[:, off:])
                                    prs[h] = (pr, off)
                                # software-pipelined PV: previous chunk
                                if prev is not None:
                                    pki, pprs = prev
                                    vsl = v_nat[:, (tb // 128) + pki, :]
                                    for h in heads:
                                        ppr, poff = pprs[h]
                                        nc.tensor.matmul(
                                            pos[h][:, poff:], vsl,
                                            ppr[:, poff:],
                                            start=(pki == 0), stop=False)
                                prev = (ki, prs)
                                drain_share()
                            # drain last PV
                            pki, pprs = prev
                            vsl = v_nat[:, (tb // 128) + pki, :]
                            for h in heads:
                                ppr, poff = pprs[h]
                                nc.tensor.matmul(pos[h][:, poff:], vsl,
                                                 ppr[:, poff:],
                                                 start=(pki == 0), stop=True)
                            drain_share()
                            # normalize: colsum+broadcast via ones-matmul,
                            # reciprocal, one multiply
                            for h in heads:
                                accb = p2n.tile([128, SQ], BF16, tag="accb")
                                nc.vector.tensor_copy(accb[:], accs[h][:])
                                denB = psS.tile([128, SQ], F32, tag="ss",
                                                name="denB")
                                nc.tensor.matmul(denB[:], ones_bf[:],
                                                 accb[:], start=True,
                                                 stop=True)
                                recS = p2n.tile([128, SQ], F32, tag="recS")
                                nc.vector.reciprocal_approx_fast(
                                    recS[:], denB[:])
                                nc.vector.tensor_mul(outT_blk[:, h, :],
                                                     pos[h][:], recS[:])
                        # queue this block's out-projection
                        tmg0 = (tb + qb * SQ) // 128
                        for t in range(4):
                            for dn in range(D // 512):
                                pending.append((outT_blk, tmg0 + t, dn,
                                                (t * 8 + dn) % 2))
                emit_outproj(len(pending), final=True)

    nc.finalize()
    return nc


_NC_CACHE = None


def _get_nc():
    global _NC_CACHE
    if _NC_CACHE is None:
        _NC_CACHE = _build_nc()
    return _NC_CACHE


def _host_tables():
    inv_freq = 1.0 / (500000.0 ** (np.arange(0, DH, 2, dtype=np.float32) / DH))
    pos = np.arange(S, dtype=np.float32)
    fr = pos[None, :] * inv_freq[:, None]            # [64, S]
    cos = np.cos(fr).astype(np.float32)
    sin = np.sin(fr).astype(np.float32)
    # feature-on-partition rope tables for [dh, tok] layout (64 distinct
    # rows; the kernel applies them to both psum halves with explicit
    # sub/add for the rotation signs)
    csf = cos                                        # [64, S]
    snf = sin                                        # [64, S]
    # triangular mask for the diagonal 128x128 block: valid iff j >= p
    j = np.arange(128)[None, :]
    p = np.arange(128)[:, None]
    m = (j >= p)
    eye = np.eye(128)
    return (csf.astype(BF), snf.astype(BF), m.astype(BF), eye.astype(BF))


def kernel(x: np.ndarray, w_qkv: np.ndarray, w_o: np.ndarray) -> np.ndarray:
    x = np.asarray(x, np.float32)
    w_qkv = np.asarray(w_qkv, np.float32)
    w_o = np.asarray(w_o, np.float32)
    nc = _get_nc()
    csf, snf, maskT, eye = _host_tables()

    xTf = x.reshape(T, D).T.astype(BF)                           # [D, T]
    # pack: xP[bb, p, k*256 + t] = xT[k*128 + p, bb*256 + t]
    xP = np.ascontiguousarray(
        xTf.reshape(KCH, 128, NB, TB).transpose(2, 1, 0, 3)
           .reshape(NB, 128, KCH * TB))
    in_maps = []
    for c in range(N_CORES):
        rows = np.concatenate([
            np.arange(4 * c * DH, (4 * c + 4) * DH),             # 4 q heads
            np.arange(H * DH + c * DH, H * DH + (c + 1) * DH),   # k head
            np.arange((H + KV) * DH + c * DH, (H + KV) * DH + (c + 1) * DH),  # v head
        ])
        wqT = np.ascontiguousarray(w_qkv[rows, :].T).astype(BF)  # [4096, 768]
        # wq2[p, g, k, c] = wqT[k*128 + p, g*128 + c]
        wq2 = np.ascontiguousarray(
            wqT.reshape(KCH, 128, NG, 128).transpose(1, 2, 0, 3))
        woT = np.ascontiguousarray(
            w_o[:, c * WO_COLS:(c + 1) * WO_COLS].T).astype(BF)  # [512, D]
        in_maps.append({
            "xP": xP, "wq2": wq2, "woT": woT,
            "csf": csf, "snf": snf, "maskT": maskT, "eye": eye,
        })

    res = run_bass_kernel_spmd(nc, in_maps, core_ids=list(range(N_CORES)))
    globals()['_LAST_RESULT'] = res
    out = np.zeros((T, D), np.float32)
    for c in range(N_CORES):
        out += res.results[c]["y"].astype(np.float32)
    return out.reshape(B, S, D)
